# revision 10
# baseline (speedup 1.0000x reference)
"""Trainium2 Bass kernel for GQA attention prefill (B=4,S=1024,D=4096,H=32,HKV=8).

Sharding (8 cores): core c owns kv-head c and query heads 4c..4c+3.
Per core: QKV projection (bf16) + RoPE + causal attention for its heads,
transposed attention output [512 hcol, 1024 t] AllGathered per batch
(bf16, 1MB per rank), then every core computes a 512-wide e-column slice
of the output projection with its resident Wo shard. Host concatenates
the 8 column slices. No AllReduce.
"""
import os
import numpy as np
import ml_dtypes

import concourse.bass as bass
import concourse.bacc as bacc
import concourse.tile as tile
from concourse.tile import add_dep_helper
import concourse.mybir as mybir
from concourse.bass_utils import run_bass_kernel_spmd

B, S, D, H, HKV, HD = 4, 1024, 4096, 32, 8, 128
NCORES = 8
QH = H // NCORES          # 4 query heads per core
DC = D // 128             # 32 contraction chunks
ESL = D // NCORES         # 512 output columns per core
T = B * S                 # 4096 tokens
F32 = mybir.dt.float32
BF16 = mybir.dt.bfloat16
BF = ml_dtypes.bfloat16

# Wo rows reordered to match the gathered head-pair halves:
# half 0 = heads (4r, 4r+1) per rank r, half 1 = heads (4r+2, 4r+3).
_rows = []
for _hf in range(QH):
    for _r in range(NCORES):
        _h = QH * _r + _hf
        _rows.append(np.arange(_h * HD, (_h + 1) * HD))
WO_ROW_PERM = np.concatenate(_rows)

LAST_EXEC_NS = None
_CACHE = {}


def _build_graph():
    nc = bacc.Bacc("TRN2", target_bir_lowering=False, debug=False, num_devices=NCORES)

    seqT_d = nc.dram_tensor("seqT", [D, T], BF16, kind="ExternalInput")
    wq_d = nc.dram_tensor("wq", [D, QH * HD], BF16, kind="ExternalInput")
    wk_d = nc.dram_tensor("wk", [D, HD], BF16, kind="ExternalInput")
    wv_d = nc.dram_tensor("wv", [D, HD], BF16, kind="ExternalInput")
    wo_d = nc.dram_tensor("wo", [D, ESL], BF16, kind="ExternalInput")
    c_d = nc.dram_tensor("ct", [HD, S], BF16, kind="ExternalInput")
    d_d = nc.dram_tensor("dt", [HD, S], BF16, kind="ExternalInput")
    id_d = nc.dram_tensor("ident", [128, 128], BF16, kind="ExternalInput")
    db_d = nc.dram_tensor("dbias", [128, 128], BF16, kind="ExternalInput")
    out_d = nc.dram_tensor("out", [T, ESL], F32, kind="ExternalOutput")

    RG = [list(range(NCORES))]

    with tile.TileContext(nc) as tc:
        with (
            tc.tile_pool(name="wpool", bufs=1) as wpool,
            tc.tile_pool(name="spool", bufs=2) as spool,
            tc.tile_pool(name="qkv", bufs=2) as qkvpool,
            tc.tile_pool(name="rp", bufs=1) as rpool,
            tc.tile_pool(name="ap", bufs=2) as apool,
            tc.tile_pool(name="atp", bufs=1) as atpool,
            tc.tile_pool(name="otp", bufs=2) as otpool,
            tc.tile_pool(name="gp", bufs=2) as gpool,
            tc.tile_pool(name="wop", bufs=1) as wopool,
            tc.tile_pool(name="scp", bufs=4) as scpool,
            tc.tile_pool(name="ps", bufs=1, space="PSUM") as ps,
            tc.tile_pool(name="dram", bufs=1, space="DRAM") as dpool,
        ):
            # ---- persistent weights/constants, batched 3D-AP DMA loads ----
            wq_all = wpool.tile([128, DC * QH * HD], BF16, tag="wq", name="wq_all")
            wq_r = wq_d.ap().rearrange("(dc p) n -> p dc n", p=128)
            wq_v = wq_all[:].rearrange("p (dc n) -> p dc n", dc=DC)
            nc.sync.dma_start(wq_v[:, 0:16, :], wq_r[:, 0:16, :])
            nc.sync.dma_start(wq_v[:, 16:32, :], wq_r[:, 16:32, :])
            wk_all = wpool.tile([128, DC * HD], BF16, tag="wk", name="wk_all")
            wv_all = wpool.tile([128, DC * HD], BF16, tag="wv", name="wv_all")

            def load_wkv():
                nc.sync.dma_start(
                    wk_all[:].rearrange("p (dc n) -> p dc n", dc=DC),
                    wk_d.ap().rearrange("(dc p) n -> p dc n", p=128),
                )
                nc.sync.dma_start(
                    wv_all[:].rearrange("p (dc n) -> p dc n", dc=DC),
                    wv_d.ap().rearrange("(dc p) n -> p dc n", p=128),
                )
            wo_all = wpool.tile([128, DC * ESL], BF16, tag="wo", name="wo_all")

            def load_wo():
                wo_r = wo_d.ap().rearrange("(hc p) e -> p hc e", p=128)
                wo_v = wo_all[:].rearrange("p (hc e) -> p hc e", hc=DC)
                nc.sync.dma_start(wo_v[:, 0:16, :], wo_r[:, 0:16, :])
                nc.sync.dma_start(wo_v[:, 16:32, :], wo_r[:, 16:32, :])

            def wq_sl(dc, h):
                return wq_all[:, dc * 512 + h * 128: dc * 512 + (h + 1) * 128]

            def wk_sl(dc):
                return wk_all[:, dc * 128:(dc + 1) * 128]

            def wv_sl(dc):
                return wv_all[:, dc * 128:(dc + 1) * 128]

            def wo_sl(hc):
                return wo_all[:, hc * 512:(hc + 1) * 512]

            c_sb = wpool.tile([HD, S], BF16, tag="c", name="c_sb")
            nc.sync.dma_start(c_sb[:], c_d.ap())
            d_sb = wpool.tile([HD, S], BF16, tag="d", name="d_sb")
            nc.sync.dma_start(d_sb[:], d_d.ap())
            id_sb = wpool.tile([128, 128], BF16, tag="id", name="id_sb")
            nc.sync.dma_start(id_sb[:], id_d.ap())
            db_sb = wpool.tile([128, 128], BF16, tag="db", name="db_sb")
            nc.sync.dma_start(db_sb[:], db_d.ap())

            seqT_r = seqT_d.ap().rearrange("(dc p) t -> p dc t", p=128)

            def rope(dst, psrc, col0, ncols):
                csl = c_sb[:, col0:col0 + ncols]
                dsl = d_sb[:, col0:col0 + ncols]
                # swapped-halves copy via ACT frees the PSUM bank quickly;
                # walrus requires SBUF tensor_tensor operands to share the
                # start partition, so the swap happens in the copy.
                t0s = rpool.tile([128, ncols], F32, tag="rt0", name="rt0", bufs=2)
                nc.scalar.copy(t0s[0:64, :], psrc[64:128, :])
                nc.scalar.copy(t0s[64:128, :], psrc[0:64, :])
                t1 = rpool.tile([128, ncols], F32, tag="rt1", name="rt1")
                t2 = rpool.tile([128, ncols], F32, tag="rt2", name="rt2")
                nc.vector.tensor_mul(t1[:], psrc[:, :], csl)
                nc.vector.tensor_mul(t2[:], t0s[:], dsl[:])
                nc.vector.tensor_add(dst[:, col0:col0 + ncols], t1[:], t2[:])

            state = {}

            def proj(b):
                q_sb = [
                    qkvpool.tile([128, S], BF16, tag=f"q{h}", name=f"q{h}b{b}")
                    for h in range(QH)
                ]
                k_sb = qkvpool.tile([128, S], BF16, tag="k", name=f"kb{b}")
                v_sb = [
                    qkvpool.tile([128, 128], BF16, tag=f"v{j}", name=f"v{j}b{b}")
                    for j in range(8)
                ]
                for qt in range(4):
                    col0 = qt * 256
                    st = spool.tile([128, DC * 256], BF16, tag="seq", name="st")
                    st_v = st[:].rearrange("p (dc t) -> p dc t", dc=DC)
                    src = seqT_r[:, :, b * S + col0: b * S + col0 + 256]
                    nc.sync.dma_start(st_v[:, 0:16, :], src[:, 0:16, :])
                    nc.sync.dma_start(st_v[:, 16:32, :], src[:, 16:32, :])
                    if b == 0 and qt == 0:
                        load_wkv()

                    def st_sl(dc):
                        return st[:, dc * 256:(dc + 1) * 256]

                    # phases: (q0,q1), (q2,q3), (K,v0 then v1)
                    for ph in range(2):
                        pa = ps.tile([128, 256], F32, tag="pjA", bufs=1, name="pa")
                        pb = ps.tile([128, 256], F32, tag="pjB", bufs=1, name="pb")
                        for dc in range(DC):
                            fl, ll = (dc == 0), (dc == DC - 1)
                            nc.tensor.matmul(pa[:], wq_sl(dc, 2 * ph), st_sl(dc), start=fl, stop=ll)
                            nc.tensor.matmul(pb[:], wq_sl(dc, 2 * ph + 1), st_sl(dc), start=fl, stop=ll)
                        rope(q_sb[2 * ph], pa[:], col0, 256)
                        rope(q_sb[2 * ph + 1], pb[:], col0, 256)
                    pa = ps.tile([128, 256], F32, tag="pjA", bufs=1, name="pa3")
                    pb = ps.tile([128, 128], F32, tag="pjB", bufs=1, name="pb3")
                    for dc in range(DC):
                        fl, ll = (dc == 0), (dc == DC - 1)
                        nc.tensor.matmul(pa[:], wk_sl(dc), st_sl(dc), start=fl, stop=ll)
                        nc.tensor.matmul(pb[:], st_sl(dc)[:, 0:128], wv_sl(dc), start=fl, stop=ll)
                    rope(k_sb, pa[:], col0, 256)
                    nc.scalar.copy(v_sb[2 * qt][:], pb[:])
                    pb = ps.tile([128, 128], F32, tag="pjB", bufs=1, name="pb4")
                    for dc in range(DC):
                        fl, ll = (dc == 0), (dc == DC - 1)
                        nc.tensor.matmul(pb[:], st_sl(dc)[:, 128:256], wv_sl(dc), start=fl, stop=ll)
                    nc.scalar.copy(v_sb[2 * qt + 1][:], pb[:])
                state[b] = {"q": q_sb, "k": k_sb, "v": v_sb}

            def attn(b):
                st_b = state[b]
                q_sb, k_sb, v_sb = st_b["q"], st_b["k"], st_b["v"]
                ot_sb = [
                    otpool.tile([128, S], BF16, tag=f"ot{h}", name=f"ot{h}b{b}")
                    for h in range(QH)
                ]
                agin = [
                    dpool.tile([128, S], BF16, tag=f"agin{b}{hf}",
                               name=f"agin{b}{hf}")
                    for hf in range(QH)
                ]
                state[b]["agin"] = agin
                sctr = 0
                for h in range(QH):
                    at_buf = [
                        atpool.tile([128, S - j * 128], BF16, tag=f"at{j}",
                                    name=f"at{j}h{h}b{b}")
                        for j in range(8)
                    ]
                    for qi in range(8):
                        width = (qi + 1) * 128
                        a_sb = apool.tile([128, S], BF16, tag="a", name="a_sb")
                        rs = []
                        for c0 in range(0, width, 512):
                            n = min(512, width - c0)
                            s_ps = ps.tile([128, 512], F32, tag=f"s{sctr % 2}",
                                           bufs=1, name="s_ps")
                            sctr += 1
                            nc.tensor.matmul(
                                s_ps[:, 0:n],
                                q_sb[h][:, qi * 128:(qi + 1) * 128],
                                k_sb[:, c0:c0 + n],
                                start=True, stop=True,
                            )
                            if c0 + n == width:  # diagonal block lives here
                                nc.vector.tensor_add(
                                    s_ps[:, n - 128:n], s_ps[:, n - 128:n], db_sb[:]
                                )
                            rsum = scpool.tile([128, 1], F32, tag="rsum",
                                               name="rsum")
                            nc.scalar.activation(
                                a_sb[:, c0:c0 + n], s_ps[:, 0:n],
                                mybir.ActivationFunctionType.Exp,
                                accum_out=rsum[:],
                            )
                            rs.append(rsum)
                        if len(rs) == 2:
                            tot = scpool.tile([128, 1], F32, tag="rtot", name="rtot")
                            nc.vector.tensor_add(tot[:], rs[0][:], rs[1][:])
                        else:
                            tot = rs[0]
                        recip = scpool.tile([128, 1], F32, tag="recip", name="recip")
                        nc.vector.reciprocal(recip[:], tot[:])
                        nc.vector.tensor_scalar_mul(
                            a_sb[:, 0:width], a_sb[:, 0:width], recip[:]
                        )
                        for j in range(qi + 1):
                            tr_ps = ps.tile([128, 128], BF16, tag="tr", bufs=2,
                                            name="tr_ps")
                            nc.tensor.transpose(
                                tr_ps[:], a_sb[:, j * 128:(j + 1) * 128], id_sb[:]
                            )
                            dst = at_buf[j][:, (qi - j) * 128:(qi - j + 1) * 128]
                            if j % 2 == 0:
                                nc.scalar.copy(dst, tr_ps[:])
                            else:
                                nc.vector.tensor_copy(dst, tr_ps[:])
                    for half in range(2):
                        q0 = half * 512
                        o_ps = ps.tile([128, 512], F32, tag="ot", bufs=1, name="o_ps")
                        jn = 4 * (half + 1)
                        for j in range(jn):
                            qlo = max(j * 128, q0)
                            nc.tensor.matmul(
                                o_ps[:, qlo - q0:512],
                                v_sb[j][:],
                                at_buf[j][:, qlo - j * 128: q0 + 512 - j * 128],
                                start=(j == 0), stop=(j == jn - 1),
                            )
                        cp = nc.scalar.copy(ot_sb[h][:, q0:q0 + 512], o_ps[:])
                        if h == 2 and half == 1:
                            state[b]["marker"] = cp
                    nc.sync.dma_start(agin[h][:, :], ot_sb[h][:])
                state[b]["ot"] = ot_sb

            def emit_ag(b, hf):
                agin = state[b]["agin"][hf]
                agout = dpool.tile(
                    [NCORES * 128, S], BF16, tag=f"agout{b}{hf}",
                    name=f"agout{b}{hf}", addr_space="Shared",
                )
                nc.gpsimd.collective_compute(
                    "AllGather", mybir.AluOpType.bypass,
                    ins=[agin.opt()], outs=[agout.opt()], replica_groups=RG,
                )
                state[b].setdefault("agout", {})[hf] = agout

            def wo_stage(b):
                marker = state.get(b + 1, {}).get("marker")
                ag_r = [
                    state[b]["agout"][hf][:, :].rearrange("(hc p) t -> p hc t", p=128)
                    for hf in range(QH)
                ]
                for tt in range(8):
                    gs = []
                    for hf in range(QH):
                        g = gpool.tile([128, 8 * 128], BF16, tag=f"g{hf}",
                                       name=f"g{hf}")
                        nc.sync.dma_start(
                            g[:].rearrange("p (hc t) -> p hc t", hc=8),
                            ag_r[hf][:, :, tt * 128:(tt + 1) * 128],
                        )
                        gs.append(g)
                    wo_ps = ps.tile([128, ESL], F32, tag="wo", bufs=1, name="wo_ps")
                    for hc in range(DC):
                        g = gs[hc // 8]
                        lc = hc % 8
                        mm = nc.tensor.matmul(
                            wo_ps[:], g[:, lc * 128:(lc + 1) * 128], wo_sl(hc),
                            start=(hc == 0), stop=(hc == DC - 1),
                        )
                        if hc == 0 and marker is not None:
                            add_dep_helper(
                                mm.ins, marker.ins, sync=False,
                                reason="keep wo after next batch attn h2",
                            )
                    osb = wopool.tile([128, ESL], F32, tag="osb", name="osb")
                    nc.vector.tensor_copy(osb[:], wo_ps[:])
                    nc.sync.dma_start(
                        out_d.ap()[b * S + tt * 128: b * S + (tt + 1) * 128, :],
                        osb[:],
                    )

            for b in range(B):
                proj(b)
                if b == 0:
                    load_wo()
                attn(b)
                if b >= 1:
                    for hf in range(QH):
                        emit_ag(b - 1, hf)
                    wo_stage(b - 1)
            for hf in range(QH):
                emit_ag(B - 1, hf)
            wo_stage(B - 1)

    nc.compile()
    return nc


def _host_prep(sequence, Wq, Wk, Wv, Wo, rope_cos, rope_sin, mask):
    seq2d = np.asarray(sequence, np.float32).reshape(T, D)
    seqT = np.ascontiguousarray(seq2d.T).astype(BF)

    perm = np.concatenate([np.arange(0, HD, 2), np.arange(1, HD, 2)])
    Wq = np.asarray(Wq, np.float32)
    Wk = np.asarray(Wk, np.float32)
    Wv = np.asarray(Wv, np.float32)
    Wo = np.asarray(Wo, np.float32)

    cosT = np.ascontiguousarray(np.asarray(rope_cos, np.float32).T)  # [64, S]
    sinT = np.ascontiguousarray(np.asarray(rope_sin, np.float32).T)
    ctile = np.concatenate([cosT, cosT], axis=0).astype(BF)
    dtile = np.concatenate([-sinT, sinT], axis=0).astype(BF)

    tril_blk = np.tril(np.ones((128, 128), dtype=bool))
    dbias = np.where(tril_blk, 0.0, -1e30).astype(BF)
    ident = np.eye(128, dtype=BF)

    scale = 1.0 / np.sqrt(HD)
    in_maps = []
    for c in range(NCORES):
        wq_c = Wq[:, c * QH * HD:(c + 1) * QH * HD].reshape(D, QH, HD)[:, :, perm]
        wq_c = (wq_c.reshape(D, QH * HD) * scale).astype(BF)
        wk_c = np.ascontiguousarray(Wk[:, c * HD:(c + 1) * HD][:, perm]).astype(BF)
        wv_c = np.ascontiguousarray(Wv[:, c * HD:(c + 1) * HD]).astype(BF)
        wo_c = np.ascontiguousarray(Wo[WO_ROW_PERM, c * ESL:(c + 1) * ESL]).astype(BF)
        in_maps.append({
            "seqT": seqT, "wq": np.ascontiguousarray(wq_c), "wk": wk_c,
            "wv": wv_c, "wo": wo_c, "ct": ctile, "dt": dtile,
            "ident": ident, "dbias": dbias,
        })
    return in_maps


def _reference_numpy(sequence, Wq, Wk, Wv, Wo, rope_cos, rope_sin, mask, start_position):
    """Pure-numpy fallback for off-spec inputs (non-tril mask etc.)."""
    seq = np.asarray(sequence, np.float32)
    Wq = np.asarray(Wq, np.float32); Wk = np.asarray(Wk, np.float32)
    Wv = np.asarray(Wv, np.float32); Wo = np.asarray(Wo, np.float32)
    cos = np.asarray(rope_cos, np.float32); sin = np.asarray(rope_sin, np.float32)
    b, s, _ = seq.shape
    L = S
    start = int(start_position)
    start = max(0, min(start, L - s))

    def rope_apply(x):
        bb, hh, ss, hd = x.shape
        xr = x.reshape(bb, hh, ss, hd // 2, 2)
        a, b2 = xr[..., 0], xr[..., 1]
        c = cos[None, None, :ss, :]
        d = sin[None, None, :ss, :]
        return np.stack([a * c - b2 * d, a * d + b2 * c], axis=-1).reshape(bb, hh, ss, hd)

    q = (seq @ Wq).reshape(b, s, H, HD).transpose(0, 2, 1, 3)
    k = (seq @ Wk).reshape(b, s, HKV, HD).transpose(0, 2, 1, 3)
    v = (seq @ Wv).reshape(b, s, HKV, HD).transpose(0, 2, 1, 3)
    q = rope_apply(q); k = rope_apply(k)
    kc = np.zeros((b, HKV, L, HD), np.float32); kc[:, :, start:start + s] = k
    vc = np.zeros((b, HKV, L, HD), np.float32); vc[:, :, start:start + s] = v
    kv_len = start + s
    k = np.repeat(kc[:, :, :kv_len], H // HKV, axis=1)
    v = np.repeat(vc[:, :, :kv_len], H // HKV, axis=1)
    logits = np.einsum("bhqd,bhkd->bhqk", q, k) / np.sqrt(HD)
    m = np.asarray(mask, bool)[None, None, :s, :kv_len]
    logits = np.where(m, logits, np.finfo(np.float32).min)
    logits -= logits.max(axis=-1, keepdims=True)
    e = np.exp(logits)
    attn = e / e.sum(axis=-1, keepdims=True)
    out = np.einsum("bhqk,bhkd->bhqd", attn, v)
    out = out.transpose(0, 2, 1, 3).reshape(b, s, H * HD)
    return (out @ Wo).astype(np.float32)


def _install_profile_shim():
    import sys, types
    if "antenv.axon_hooks" in sys.modules:
        return
    mod = types.ModuleType("antenv.axon_hooks")
    state = {"hook": None}
    mod.set_axon_ntff_profile_hook = lambda h: state.__setitem__("hook", h)
    mod.get_axon_ntff_profile_hook = lambda: state["hook"]
    sys.modules["antenv.axon_hooks"] = mod
    import antenv
    antenv.axon_hooks = mod
    try:
        from trn_agent_boot.trn_boot import _ntff_profile_via_ctypes
        hook = _ntff_profile_via_ctypes("/opt/axon/libaxon_pjrt.so")
        if hook is not None:
            mod.set_axon_ntff_profile_hook(hook)
    except Exception:
        pass


def kernel(sequence, Wq, Wk, Wv, Wo, rope_cos, rope_sin, mask, start_position):
    global LAST_EXEC_NS
    sequence = np.asarray(sequence)
    mask_np = np.asarray(mask, bool)
    tril = np.tril(np.ones((S, S), dtype=bool))
    if (
        sequence.shape != (B, S, D)
        or int(start_position) != 0
        or mask_np.shape != (S, S)
        or not np.array_equal(mask_np, tril)
    ):
        return _reference_numpy(
            sequence, Wq, Wk, Wv, Wo, rope_cos, rope_sin, mask, start_position
        )

    in_maps = _host_prep(sequence, Wq, Wk, Wv, Wo, rope_cos, rope_sin, mask)
    if "nc" not in _CACHE:
        _CACHE["nc"] = _build_graph()
    nc = _CACHE["nc"]

    trace = bool(os.environ.get("BASS_KERNEL_PROFILE"))
    if trace:
        _install_profile_shim()
    try:
        res = run_bass_kernel_spmd(nc, in_maps, list(range(NCORES)), trace=trace)
    except Exception:
        if not trace:
            raise
        res = run_bass_kernel_spmd(nc, in_maps, list(range(NCORES)), trace=False)
    LAST_EXEC_NS = res.exec_time_ns

    outs = [np.asarray(res.results[c]["out"], np.float32) for c in range(NCORES)]
    full = np.concatenate(outs, axis=1)  # [T, D]
    return full.reshape(B, S, D)
